# revision 1
# baseline (speedup 1.0000x reference)
"""Contextual patches score kernel for Trainium2 (8 NeuronCores).

Computes, per sample i:
    fs = f[i, :, ::2, ::2]; bs = b[i, :, ::2, ::2]          # [64, 80, 80]
    w  = 3x3 patches of bs (SAME, stride 1)                  # [6400, 64, 3, 3]
    wn = w / max(||w||_2, 1e-4)
    y[i] = conv(fs, wn, SAME)                                # [6400, 80, 80]

Implementation: y[l, p] = (w_l . f_patch_p) * inv_norm_l is a
[6400, 576] x [576, 6400] matmul per sample.  Sharding: 8 cores =
2 samples x 4 spatial-row quarters; each core computes [6400, 1600].
K = 576 = 64 channels x 9 taps, packed as 5 chunks of 128 partitions
(tap pairs stacked; last chunk zero-padded).  Operands are built once
in SBUF by copying shifted windows out of zero-padded images (a
row-shifted replica of each image lives in partitions 64-127 so a tap
pair is a single lane-aligned copy); fine-grained operand tiles let
the matmul stream start after ~2 tiles of build.  float32r matmuls
(full-rate at moving-dim >= 256, ~1e-4 rel err).  Patch normalization
is applied as a per-output-row scale after PSUM accumulation.
Patch norms: ACT squares the weight chunks, DVE sums them in F32
(f32r-input DVE arithmetic is silently wrong on this HW; the final add
writes an F32R tile), and a single ones-matmul per m-tile does the
partition reduction; the first 8 m-tiles use 5 ones-matmuls instead so
the DVE adds stay out of the build-congested startup ramp.
Measured ~230us/core on TRN2 (rel err 1.45e-4; PE-bound at ~199us).
"""

import numpy as np

import concourse.bass as bass
import concourse.mybir as mybir
import concourse.tile as tile
from concourse.bass_utils import run_bass_kernel_spmd

F32 = mybir.dt.float32
F32R = mybir.dt.float32r
AF = mybir.ActivationFunctionType

C = 64            # channels
H = W = 80        # downsampled spatial size
L = H * W         # 6400 patches per sample
QROWS = 20        # output rows handled per core
POS = QROWS * W   # 1600 output positions per core
NTILE = 400       # matmul moving free dim (5 rows x 80)
NT = POS // NTILE         # 4 n-tiles
MT = L // 128             # 50 m-tiles
HALF_MT = MT // 2         # 25 (lhsT is split in two halves for pipelining)
NCHUNK = 5                # K chunks: 4 full tap pairs + 1 half (tap 8)
EPS = 1e-4

# chunk -> ((kh, kw) for partitions 0:64, (kh, kw) for partitions 64:128)
# The replica half of each padded image is shifted up one row, so a
# (kh, kw) / (kh+1, kw') pair reads with a single AP offset per half.
_CHUNK_TAPS = [
    ((0, 0), (1, 0)),
    ((0, 1), (1, 1)),
    ((0, 2), (1, 2)),
    ((2, 0), (2, 1)),
    ((2, 2), None),
]


def _win(img, kh, kw, nrows):
    """[*, nrows, 80] shifted window of a padded [*, rows, 82] image tile."""
    return img[:, kh:kh + nrows, kw:kw + W]


_COPY_SEQ = [0]


def _copy_chunks(nc, dst3, img, nrows):
    """Fill dst3 [128, 5, nrows*80] with the 5 K-chunks of im2col windows.

    img: [128, nrows+2, 82] padded image; partitions 64:128 hold the
    same image shifted up one row (img2[c, r, x] = img1[c, r+1, x]).
    """
    def dst(j, p0, p1):
        return dst3[p0:p1, j, :].rearrange("p (y x) -> p y x", x=W)

    def copy(out, in_):
        # DVE:ACT = 2:1 (ACT copies are ~2x slower; this balances the
        # two engines so the serial setup phase is roughly halved).
        # The first 14 copies (rhs0 + lhsT0, which gate the first
        # matmuls) stay DVE-only: ACT starts ~3.5us late (cold boot).
        i = _COPY_SEQ[0]
        _COPY_SEQ[0] += 1
        if i < 14 or i % 3 != 2:
            nc.vector.tensor_copy(out, in_)
        else:
            nc.scalar.activation(out, in_, AF.Copy)

    for j in range(3):
        (kh, kw), _ = _CHUNK_TAPS[j]
        copy(dst(j, 0, 128), _win(img, kh, kw, nrows))
    # chunk 3: tap (2,0) from base half, tap (2,1) via replica (kh-1 index)
    copy(dst(3, 0, 64), _win(img[0:64], 2, 0, nrows))
    copy(dst(3, 64, 128), _win(img[64:128], 1, 1, nrows))
    # chunk 4: tap (2,2); upper partitions stay zero (memset at caller —
    # K=64 / tile_position-paired fp32r matmuls measured 2x SLOWER than
    # a full zero-padded K=128 matmul, so pad instead of packing)
    copy(dst(4, 0, 64), _win(img[0:64], 2, 2, nrows))


def build_nc():
    _COPY_SEQ[0] = 0
    nc = bass.Bass(target_bir_lowering=False)
    fs_d = nc.dram_tensor("fs_pad", [C, QROWS + 2, 82], F32, kind="ExternalInput")
    bs_d = nc.dram_tensor("bs_pad", [C, 82, 82], F32, kind="ExternalInput")
    y_d = nc.dram_tensor("y", [L, POS], F32, kind="ExternalOutput")

    with tile.TileContext(nc) as tc:
        with (
            tc.tile_pool(name="big", bufs=1) as big,
            tc.tile_pool(name="pad", bufs=2) as padp,
            tc.tile_pool(name="sq", bufs=2) as sqp,
            tc.tile_pool(name="inv", bufs=4) as invp,
            tc.tile_pool(name="outp", bufs=3) as outp,
            tc.tile_pool(name="ps", bufs=6, space="PSUM") as psp,
            tc.tile_pool(name="pss", bufs=2, space="PSUM") as pssp,
        ):
            ones = big.tile([128, 2], F32R, tag="ones")
            nc.vector.memset(ones[:].bitcast(F32), 1.0)

            # f image quarter + row-shifted replica in partitions 64:128
            fpad = big.tile([128, QROWS + 2, 82], F32, tag="fpad")
            nc.sync.dma_start(fpad[0:64, 0:11], fs_d[:, 0:11])
            nc.sync.dma_start(fpad[0:64, 11:QROWS + 2], fs_d[:, 11:QROWS + 2])
            nc.sync.dma_start(fpad[64:128, 0:11], fs_d[:, 1:12])
            nc.sync.dma_start(fpad[64:128, 11:QROWS + 1], fs_d[:, 12:QROWS + 2])

            # rhs: im2col of the f quarter, one [128, 5, 800] tile per
            # n-tile pair.  lhsT: b patches (transposed weights) in
            # [128, 5, 640] tiles (lcm(80,128): 8 image rows = exactly 5
            # m-tiles each).  Build order rhs0, lhsT0, rhs1, lhsT1.. so
            # the first matmuls gate on ~2 tiles' worth of copies and the
            # rest of the build overlaps the matmul stream.
            # allocate all operand tiles up front; zero the chunk-4 pad
            # rows on DVE for the two tiles that gate the first matmuls,
            # on the (otherwise idle) GPSIMD engine for the rest
            rhs = [big.tile([128, NCHUNK, POS // 2], F32R, tag=f"rhs{u}",
                            name=f"rhs{u}") for u in range(2)]
            lhsT = [big.tile([128, NCHUNK, 640], F32R, tag=f"lhsT{t}",
                             name=f"lhsT{t}") for t in range(MT // 5)]
            nc.vector.memset(rhs[0][64:128, 4, :].bitcast(F32), 0.0)
            nc.vector.memset(lhsT[0][64:128, 4, :].bitcast(F32), 0.0)
            nc.gpsimd.memset(rhs[1][64:128, 4, :].bitcast(F32), 0.0)
            for t in range(1, MT // 5):
                nc.gpsimd.memset(lhsT[t][64:128, 4, :].bitcast(F32), 0.0)

            def build_rhs(u):
                _copy_chunks(nc, rhs[u], fpad[:, 10 * u:10 * u + 12, :],
                             QROWS // 2)

            def build_lhsT(t):
                bt = padp.tile([128, 10, 82], F32, tag="bpad")
                nc.sync.dma_start(bt[0:64], bs_d[:, 8 * t:8 * t + 10])
                nc.sync.dma_start(
                    bt[64:128, 0:9], bs_d[:, 8 * t + 1:8 * t + 10]
                )
                _copy_chunks(nc, lhsT[t], bt, 8)

            build_rhs(0)
            build_lhsT(0)
            build_rhs(1)
            for t in range(1, MT // 5):
                build_lhsT(t)

            def norm_group(t, msl, use_adds=True):
                # inv_norm for these 128 patches: square on ACT, sum the 5
                # K-chunks elementwise on DVE (slack engine), then a SINGLE
                # ones-matmul does the 128-partition reduction -- 1 PE op
                # per m-tile instead of 5.  During the startup ramp
                # (use_adds=False) DVE is congested with operand builds, so
                # spend idle PE on 5 ones-matmuls instead of DVE adds.
                sq = sqp.tile([128, NCHUNK, 128], F32R, tag="sq")
                nc.scalar.activation(sq[:], lhsT[t][:, :, msl], AF.Square)
                ps_s = pssp.tile([128, 2], F32, tag="pss")
                if use_adds:
                    ssum = sqp.tile([128, 128], F32, tag="ssum")

                    def sqf(j):
                        return sq[:, j, :].bitcast(F32)

                    nc.vector.tensor_add(ssum[:], sqf(0), sqf(1))
                    nc.vector.tensor_add(ssum[:], ssum[:], sqf(2))
                    nc.vector.tensor_add(ssum[:], ssum[:], sqf(3))
                    # final add writes the f32r tile directly (f32 inputs
                    # are fine; f32r INPUTS to DVE arithmetic are wrong)
                    ssr = sqp.tile([128, 128], F32R, tag="ssr")
                    nc.vector.tensor_add(ssr[:], ssum[:], sqf(4))
                    nc.tensor.matmul(
                        ps_s[:], lhsT=ssr[:], rhs=ones[:],
                        start=True, stop=True,
                    )
                else:
                    for j in range(NCHUNK):
                        nc.tensor.matmul(
                            ps_s[:],
                            lhsT=sq[:, j, :],
                            rhs=ones[:],
                            start=(j == 0),
                            stop=(j == NCHUNK - 1),
                        )
                inv = invp.tile([128, 1], F32, tag="inv")
                nc.scalar.activation(inv[:], ps_s[:, 0:1], AF.Sqrt)
                nc.vector.tensor_scalar(
                    inv[:], inv[:], EPS, None, mybir.AluOpType.max
                )
                nc.vector.reciprocal(inv[:], inv[:])
                return inv

            for m in range(MT):
                t, ml = divmod(m, 5)
                msl = slice(ml * 128, (ml + 1) * 128)
                last = m == MT - 1
                tail_dma = m >= MT - 1

                # main matmuls first (keeps PE start independent of the
                # ACT Square in the norm group); for the last m-tile the
                # norm group goes first instead, to shorten the tail
                if last:
                    inv = norm_group(t, msl, use_adds=m >= 8)
                pstiles = []
                for nt in range(NT):
                    ps = psp.tile([128, NTILE], F32, tag="ps")
                    pstiles.append(ps)
                    for j in range(NCHUNK):
                        nc.tensor.matmul(
                            ps[:],
                            lhsT=lhsT[t][:, j, msl],
                            rhs=rhs[nt // 2][:, j,
                                            (nt % 2) * NTILE:(nt % 2 + 1) * NTILE],
                            start=(j == 0),
                            stop=(j == NCHUNK - 1),
                        )
                if not last:
                    inv = norm_group(t, msl, use_adds=m >= 8)

                # n-tiles in pairs sharing one [128, 800] output staging
                # tile -> one DMA per pair (halves Sync-sequencer issues).
                # The last m-tile instead issues 4 half-partition DMAs per
                # pair: a single [128, 800] DMA occupies one HW queue for
                # ~10us, which would otherwise be the kernel tail.
                for nt0 in range(0, NT, 2):
                    ot = outp.tile([128, 2, NTILE], F32, tag="ot")
                    for i, nt in enumerate((nt0, nt0 + 1)):
                        # alternate DVE / ACT: after the norm rework both
                        # PE and DVE sit at ~198us while ACT has ~125us
                        # slack; splitting the 200 scale-copies rebalances
                        # DVE and ACT to ~160us each
                        if i == 0:
                            nc.vector.tensor_scalar_mul(
                                ot[:, i, :], pstiles[nt][:], inv[:]
                            )
                        else:
                            nc.scalar.activation(
                                ot[:, i, :], pstiles[nt][:], AF.Copy,
                                scale=inv[:],
                            )
                        if tail_dma:
                            for p0 in (0, 64):
                                nc.sync.dma_start(
                                    y_d[m * 128 + p0:m * 128 + p0 + 64,
                                        nt * NTILE:(nt + 1) * NTILE],
                                    ot[p0:p0 + 64, i, :],
                                )
                    if not tail_dma:
                        nc.sync.dma_start(
                            y_d[m * 128:(m + 1) * 128,
                                nt0 * NTILE:(nt0 + 2) * NTILE],
                            ot[:],
                        )
    return nc


def _split_multiwaits(nc, maxw=1):
    """Walrus (this build) accepts at most one sync-wait per instruction.

    Tile's kernel-tail drain carries one wait per active logical proc, so
    hoist excess waits onto same-engine NoOps inserted right before the
    offending instruction (engine executes them in order -> identical
    blocking semantics)."""
    n = 0
    for fn in nc.m.functions:
        for blk in fn.blocks:
            insts = list(blk.instructions)
            new, changed = [], False
            for ins in insts:
                si = ins.sync_info
                if si is not None and len(si.on_wait) > maxw:
                    extra, keep = si.on_wait[:-maxw], si.on_wait[-maxw:]
                    k = 0
                    while extra:
                        chunk, extra = extra[:maxw], extra[maxw:]
                        new.append(mybir.InstNoOp(
                            name=f"{ins.name}-ws{k}",
                            engine=ins.engine,
                            bass_nofuse=True,
                            sync_info=mybir.SyncInfo(
                                on_wait=list(chunk), on_update=[]
                            ),
                        ))
                        k += 1
                        n += 1
                    ins.sync_info = mybir.SyncInfo(
                        on_wait=list(keep), on_update=list(si.on_update)
                    )
                    changed = True
                new.append(ins)
            if changed:
                blk.instructions = new
    return n


_CACHE = {}


def _get_nc():
    if "nc" not in _CACHE:
        nc = build_nc()
        _split_multiwaits(nc)
        _CACHE["nc"] = nc
    return _CACHE["nc"]


def make_in_maps(f, b):
    f = np.asarray(f, dtype=np.float32)
    b = np.asarray(b, dtype=np.float32)
    n_samples = f.shape[0]
    fs = f[:, :, ::2, ::2]
    bs = b[:, :, ::2, ::2]
    fpad = np.zeros((n_samples, C, 82, 82), np.float32)
    fpad[:, :, 1:81, 1:81] = fs
    bpad = np.zeros((n_samples, C, 82, 82), np.float32)
    bpad[:, :, 1:81, 1:81] = bs
    in_maps = []
    for c in range(8):
        n, q = divmod(c, 4)
        in_maps.append({
            "fs_pad": np.ascontiguousarray(fpad[n, :, 20 * q:20 * q + 22, :]),
            "bs_pad": np.ascontiguousarray(bpad[n]),
        })
    return in_maps


def assemble(results, n_samples=2):
    out = np.empty((n_samples, L, H, W), np.float32)
    for c in range(8):
        n, q = divmod(c, 4)
        out[n, :, 20 * q:20 * q + 20, :] = results[c]["y"].reshape(L, QROWS, W)
    return out


def run(f, b, **kw):
    res = run_bass_kernel_spmd(_get_nc(), make_in_maps(f, b), list(range(8)), **kw)
    return assemble(res.results, np.asarray(f).shape[0]), res


def kernel(f, b):
    out, _ = run(f, b)
    return out



# revision 9
# speedup vs baseline: 1.1028x; 1.1028x over previous
"""Contextual patches score kernel for Trainium2 (8 NeuronCores).

Computes, per sample i:
    fs = f[i, :, ::2, ::2]; bs = b[i, :, ::2, ::2]          # [64, 80, 80]
    w  = 3x3 patches of bs (SAME, stride 1)                  # [6400, 64, 3, 3]
    wn = w / max(||w||_2, 1e-4)
    y[i] = conv(fs, wn, SAME)                                # [6400, 80, 80]

Implementation: y[l, p] = (w_l . f_patch_p) * inv_norm_l is a
[6400, 576] x [576, 6400] matmul per sample.  Sharding: 8 cores =
2 samples x 4 spatial-row quarters; each core computes [6400, 1600].
K = 576 = 64 channels x 9 taps, packed as 5 chunks of 128 partitions
(tap pairs stacked; last chunk zero-padded).  Operands are built once
in SBUF by copying shifted windows out of zero-padded images (a
row-shifted replica of each image lives in partitions 64-127 so a tap
pair is a single lane-aligned copy); fine-grained operand tiles let
the matmul stream start after ~2 tiles of build.  float32r matmuls
(full-rate at moving-dim >= 256, ~1e-4 rel err).  Patch normalization
is applied as a per-output-row scale after PSUM accumulation.
Patch norms: ACT squares the weight chunks, DVE sums them in F32
(f32r-input DVE arithmetic is silently wrong on this HW; the final add
writes an F32R tile), and a single ones-matmul per m-tile does the
partition reduction; the first 8 m-tiles use 5 ones-matmuls instead so
the DVE adds stay out of the build-congested startup ramp.
Measured ~230us/core on TRN2 (rel err 1.45e-4; PE-bound at ~199us).
"""

import ml_dtypes
import numpy as np

import concourse.bass as bass
import concourse.mybir as mybir
import concourse.tile as tile
from concourse.bass_utils import run_bass_kernel_spmd

F32 = mybir.dt.float32
F32R = mybir.dt.float32r
BF16 = mybir.dt.bfloat16
AF = mybir.ActivationFunctionType
NP_BF16 = ml_dtypes.bfloat16

C = 64            # channels
H = W = 80        # downsampled spatial size
L = H * W         # 6400 patches per sample
QROWS = 20        # output rows handled per core
POS = QROWS * W   # 1600 output positions per core
NTILE = 400       # matmul moving free dim (5 rows x 80)
NT = POS // NTILE         # 4 n-tiles
MT = L // 128             # 50 m-tiles
HALF_MT = MT // 2         # 25 (lhsT is split in two halves for pipelining)
NCHUNK = 5                # K chunks: 4 full tap pairs + 1 half (tap 8)
EPS = 1e-4

# chunk -> ((kh, kw) for partitions 0:64, (kh, kw) for partitions 64:128)
# The replica half of each padded image is shifted up one row, so a
# (kh, kw) / (kh+1, kw') pair reads with a single AP offset per half.
_CHUNK_TAPS = [
    ((0, 0), (1, 0)),
    ((0, 1), (1, 1)),
    ((0, 2), (1, 2)),
    ((2, 0), (2, 1)),
    ((2, 2), None),
]


def _win(img, kh, kw, nrows):
    """[*, nrows, 80] shifted window of a padded [*, rows, 82] image tile."""
    return img[:, kh:kh + nrows, kw:kw + W]


_COPY_SEQ = [0]


def _copy_chunks(nc, dst3, img, nrows):
    """Fill dst3 [128, 5, nrows*80] with the 5 K-chunks of im2col windows.

    img: [128, nrows+2, 82] padded image; partitions 64:128 hold the
    same image shifted up one row (img2[c, r, x] = img1[c, r+1, x]).
    """
    def dst(j, p0, p1):
        return dst3[p0:p1, j, :].rearrange("p (y x) -> p y x", x=W)

    def copy(out, in_):
        # DVE:ACT = 2:1 (ACT copies are ~2x slower; this balances the
        # two engines so the serial setup phase is roughly halved).
        # The first 14 copies (rhs0 + lhsT0, which gate the first
        # matmuls) stay DVE-only: ACT starts ~3.5us late (cold boot).
        i = _COPY_SEQ[0]
        _COPY_SEQ[0] += 1
        if i < 14 or i % 3 != 2:
            nc.vector.tensor_copy(out, in_)
        else:
            nc.scalar.activation(out, in_, AF.Copy)

    for j in range(3):
        (kh, kw), _ = _CHUNK_TAPS[j]
        copy(dst(j, 0, 128), _win(img, kh, kw, nrows))
    # chunk 3: tap (2,0) from base half, tap (2,1) via replica (kh-1 index)
    copy(dst(3, 0, 64), _win(img[0:64], 2, 0, nrows))
    copy(dst(3, 64, 128), _win(img[64:128], 1, 1, nrows))
    # chunk 4: tap (2,2); upper partitions stay zero (memset at caller —
    # K=64 / tile_position-paired fp32r matmuls measured 2x SLOWER than
    # a full zero-padded K=128 matmul, so pad instead of packing)
    copy(dst(4, 0, 64), _win(img[0:64], 2, 2, nrows))


def build_nc():
    _COPY_SEQ[0] = 0
    nc = bass.Bass(target_bir_lowering=False)
    fs_d = nc.dram_tensor("fs_pad", [C, QROWS + 2, 82], BF16, kind="ExternalInput")
    bs_d = nc.dram_tensor("bs_pad", [C, 82, 82], BF16, kind="ExternalInput")
    y_d = nc.dram_tensor("y", [L, POS], BF16, kind="ExternalOutput")

    with tile.TileContext(nc) as tc:
        with (
            tc.tile_pool(name="big", bufs=1) as big,
            tc.tile_pool(name="pad", bufs=2) as padp,
            tc.tile_pool(name="sq", bufs=2) as sqp,
            tc.tile_pool(name="inv", bufs=4) as invp,
            tc.tile_pool(name="outp", bufs=3) as outp,
            tc.tile_pool(name="ps", bufs=6, space="PSUM") as psp,
            tc.tile_pool(name="pss", bufs=2, space="PSUM") as pssp,
        ):
            ones = big.tile([128, 2], F32R, tag="ones")
            nc.vector.memset(ones[:].bitcast(F32), 1.0)

            # f image quarter + row-shifted replica in partitions 64:128
            fpad = big.tile([128, QROWS + 2, 82], BF16, tag="fpad")
            nc.sync.dma_start(fpad[0:64, 0:11], fs_d[:, 0:11])
            nc.sync.dma_start(fpad[0:64, 11:QROWS + 2], fs_d[:, 11:QROWS + 2])
            nc.sync.dma_start(fpad[64:128, 0:11], fs_d[:, 1:12])
            nc.sync.dma_start(fpad[64:128, 11:QROWS + 1], fs_d[:, 12:QROWS + 2])

            # rhs: im2col of the f quarter, one [128, 5, 800] tile per
            # n-tile pair.  lhsT: b patches (transposed weights) in
            # [128, 5, 640] tiles (lcm(80,128): 8 image rows = exactly 5
            # m-tiles each).  Build order rhs0, lhsT0, rhs1, lhsT1.. so
            # the first matmuls gate on ~2 tiles' worth of copies and the
            # rest of the build overlaps the matmul stream.
            # allocate all operand tiles up front; zero the chunk-4 pad
            # rows on DVE for the two tiles that gate the first matmuls,
            # on the (otherwise idle) GPSIMD engine for the rest
            rhs = [big.tile([128, NCHUNK, POS // 2], BF16, tag=f"rhs{u}",
                            name=f"rhs{u}") for u in range(2)]
            lhsT = [big.tile([128, NCHUNK, 640], BF16, tag=f"lhsT{t}",
                             name=f"lhsT{t}") for t in range(MT // 5)]
            nc.vector.memset(rhs[0][64:128, 4, :], 0.0)
            nc.vector.memset(lhsT[0][64:128, 4, :], 0.0)
            nc.gpsimd.memset(rhs[1][64:128, 4, :], 0.0)
            for t in range(1, MT // 5):
                nc.gpsimd.memset(lhsT[t][64:128, 4, :], 0.0)

            def build_rhs(u):
                _copy_chunks(nc, rhs[u], fpad[:, 10 * u:10 * u + 12, :],
                             QROWS // 2)

            def build_lhsT(t):
                bt = padp.tile([128, 10, 82], BF16, tag="bpad")
                nc.sync.dma_start(bt[0:64], bs_d[:, 8 * t:8 * t + 10])
                nc.sync.dma_start(
                    bt[64:128, 0:9], bs_d[:, 8 * t + 1:8 * t + 10]
                )
                _copy_chunks(nc, lhsT[t], bt, 8)

            build_rhs(0)
            build_lhsT(0)
            build_rhs(1)
            for t in range(1, MT // 5):
                build_lhsT(t)

            def norm_group(t, msl, use_adds=True):
                # inv_norm for these 128 patches: square on ACT, sum the 5
                # K-chunks elementwise on DVE (slack engine), then a SINGLE
                # ones-matmul does the 128-partition reduction -- 1 PE op
                # per m-tile instead of 5.  During the startup ramp
                # (use_adds=False) DVE is congested with operand builds, so
                # spend idle PE on 5 ones-matmuls instead of DVE adds.
                sq = sqp.tile([128, NCHUNK, 128], F32R, tag="sq")
                nc.scalar.activation(sq[:], lhsT[t][:, :, msl], AF.Square)
                ps_s = pssp.tile([128, 2], F32, tag="pss")
                if use_adds:
                    ssum = sqp.tile([128, 128], F32, tag="ssum")

                    def sqf(j):
                        return sq[:, j, :].bitcast(F32)

                    nc.vector.tensor_add(ssum[:], sqf(0), sqf(1))
                    nc.vector.tensor_add(ssum[:], ssum[:], sqf(2))
                    nc.vector.tensor_add(ssum[:], ssum[:], sqf(3))
                    # final add writes the f32r tile directly (f32 inputs
                    # are fine; f32r INPUTS to DVE arithmetic are wrong)
                    ssr = sqp.tile([128, 128], F32R, tag="ssr")
                    nc.vector.tensor_add(ssr[:], ssum[:], sqf(4))
                    nc.tensor.matmul(
                        ps_s[:], lhsT=ssr[:], rhs=ones[:],
                        start=True, stop=True,
                    )
                else:
                    for j in range(NCHUNK):
                        nc.tensor.matmul(
                            ps_s[:],
                            lhsT=sq[:, j, :],
                            rhs=ones[:],
                            start=(j == 0),
                            stop=(j == NCHUNK - 1),
                        )
                inv = invp.tile([128, 1], F32, tag="inv")
                nc.scalar.activation(inv[:], ps_s[:, 0:1], AF.Sqrt)
                nc.vector.tensor_scalar(
                    inv[:], inv[:], EPS, None, mybir.AluOpType.max
                )
                nc.vector.reciprocal(inv[:], inv[:])
                return inv

            for m in range(MT):
                t, ml = divmod(m, 5)
                msl = slice(ml * 128, (ml + 1) * 128)
                last = m == MT - 1
                tail_dma = m >= MT - 1

                # main matmuls first (keeps PE start independent of the
                # ACT Square in the norm group); for the last m-tile the
                # norm group goes first instead, to shorten the tail
                if last:
                    inv = norm_group(t, msl, use_adds=m >= 8)
                pstiles = []
                for nt in range(NT):
                    ps = psp.tile([128, NTILE], F32, tag="ps")
                    pstiles.append(ps)
                    for j in range(NCHUNK):
                        nc.tensor.matmul(
                            ps[:],
                            lhsT=lhsT[t][:, j, msl],
                            rhs=rhs[nt // 2][:, j,
                                            (nt % 2) * NTILE:(nt % 2 + 1) * NTILE],
                            start=(j == 0),
                            stop=(j == NCHUNK - 1),
                        )
                if not last:
                    inv = norm_group(t, msl, use_adds=m >= 8)

                # n-tiles in pairs sharing one [128, 800] output staging
                # tile -> one DMA per pair (halves Sync-sequencer issues).
                # The last m-tile instead issues 4 half-partition DMAs per
                # pair: a single [128, 800] DMA occupies one HW queue for
                # ~10us, which would otherwise be the kernel tail.
                for nt0 in range(0, NT, 2):
                    ot = outp.tile([128, 2, NTILE], BF16, tag="ot")
                    for i, nt in enumerate((nt0, nt0 + 1)):
                        # alternate DVE / ACT: after the norm rework both
                        # PE and DVE sit at ~198us while ACT has ~125us
                        # slack; splitting the 200 scale-copies rebalances
                        # DVE and ACT to ~160us each
                        if i == 0:
                            nc.vector.tensor_scalar_mul(
                                ot[:, i, :], pstiles[nt][:], inv[:]
                            )
                        else:
                            nc.scalar.activation(
                                ot[:, i, :], pstiles[nt][:], AF.Copy,
                                scale=inv[:],
                            )
                        if tail_dma:
                            for p0 in (0, 64):
                                nc.sync.dma_start(
                                    y_d[m * 128 + p0:m * 128 + p0 + 64,
                                        nt * NTILE:(nt + 1) * NTILE],
                                    ot[p0:p0 + 64, i, :],
                                )
                    if not tail_dma:
                        nc.sync.dma_start(
                            y_d[m * 128:(m + 1) * 128,
                                nt0 * NTILE:(nt0 + 2) * NTILE],
                            ot[:],
                        )
    return nc


def _split_multiwaits(nc, maxw=1):
    """Walrus (this build) accepts at most one sync-wait per instruction.

    Tile's kernel-tail drain carries one wait per active logical proc, so
    hoist excess waits onto same-engine NoOps inserted right before the
    offending instruction (engine executes them in order -> identical
    blocking semantics)."""
    n = 0
    for fn in nc.m.functions:
        for blk in fn.blocks:
            insts = list(blk.instructions)
            new, changed = [], False
            for ins in insts:
                si = ins.sync_info
                if si is not None and len(si.on_wait) > maxw:
                    extra, keep = si.on_wait[:-maxw], si.on_wait[-maxw:]
                    k = 0
                    while extra:
                        chunk, extra = extra[:maxw], extra[maxw:]
                        new.append(mybir.InstNoOp(
                            name=f"{ins.name}-ws{k}",
                            engine=ins.engine,
                            bass_nofuse=True,
                            sync_info=mybir.SyncInfo(
                                on_wait=list(chunk), on_update=[]
                            ),
                        ))
                        k += 1
                        n += 1
                    ins.sync_info = mybir.SyncInfo(
                        on_wait=list(keep), on_update=list(si.on_update)
                    )
                    changed = True
                new.append(ins)
            if changed:
                blk.instructions = new
    return n


_CACHE = {}


def _get_nc():
    if "nc" not in _CACHE:
        nc = build_nc()
        _split_multiwaits(nc)
        _CACHE["nc"] = nc
    return _CACHE["nc"]


def make_in_maps(f, b):
    f = np.asarray(f, dtype=np.float32)
    b = np.asarray(b, dtype=np.float32)
    n_samples = f.shape[0]
    fs = f[:, :, ::2, ::2].astype(NP_BF16)
    bs = b[:, :, ::2, ::2].astype(NP_BF16)
    fpad = np.zeros((n_samples, C, 82, 82), NP_BF16)
    fpad[:, :, 1:81, 1:81] = fs
    bpad = np.zeros((n_samples, C, 82, 82), NP_BF16)
    bpad[:, :, 1:81, 1:81] = bs
    in_maps = []
    for c in range(8):
        n, q = divmod(c, 4)
        in_maps.append({
            "fs_pad": np.ascontiguousarray(fpad[n, :, 20 * q:20 * q + 22, :]),
            "bs_pad": np.ascontiguousarray(bpad[n]),
        })
    return in_maps


def assemble(results, n_samples=2):
    out = np.empty((n_samples, L, H, W), np.float32)
    for c in range(8):
        n, q = divmod(c, 4)
        out[n, :, 20 * q:20 * q + 20, :] = (
            results[c]["y"].astype(np.float32).reshape(L, QROWS, W)
        )
    return out


def run(f, b, **kw):
    res = run_bass_kernel_spmd(_get_nc(), make_in_maps(f, b), list(range(8)), **kw)
    return assemble(res.results, np.asarray(f).shape[0]), res


def kernel(f, b):
    out, _ = run(f, b)
    return out

